# revision 3
# baseline (speedup 1.0000x reference)
"""Q8 linear layer (dequant matmul) on 8 Trainium2 NeuronCores.

out[t, o] = sum_i (x[t, i] * scales[i]) * weight[o, i]

Sharding: tensor-parallel over out_features (14336 = 8 * 1792). Each core
receives the full (pre-scaled, pre-transposed) activations and a 1792-column
slice of weight^T in bf16 (int8-valued weights are exact in bf16), computes
its [32, 1792] f32 output slice, and the host concatenates.

Device kernel per core:
  - preload x_scaled^T as 32 k-tiles of [128, 32] bf16 (stationary operand)
  - stream weight^T k-tiles [128, 1792] bf16 (moving operand)
  - accumulate out^T blocks in 4 PSUM banks over the 32 k-tiles
  - copy PSUM -> SBUF f32, one DMA back to HBM
"""

import os
import sys

for _p in ("/opt/trn_rl_repo", "/root/.axon_site/_ro/trn_rl_repo"):
    if os.path.isdir(_p) and _p not in sys.path:
        sys.path.insert(0, _p)

import numpy as np
import ml_dtypes

import concourse.bass as bass
import concourse.mybir as mybir
import concourse.tile as tile
from concourse import bacc
from concourse.bass_utils import run_bass_kernel_spmd

TOKENS = 32
IN_F = 4096
OUT_F = 14336
NCORES = 8
OPC = OUT_F // NCORES  # 1792 out features per core
KT = IN_F // 128  # 32 k-tiles
OB = 4  # output column blocks per core
OBS = OPC // OB  # 448 columns per block (fits one PSUM bank)

_cached_nc = {}


def _emit_body(nc, tc, pools, aps, it=0):
    xpool, wpool, opool, pspool = pools
    xsT_r, wT_r, out = aps

    xs_sb = xpool.tile(
        [128, KT, TOKENS], mybir.dt.bfloat16, name=f"xs_sb_{it}", tag="xs_sb"
    )
    nc.sync.dma_start(out=xs_sb[:], in_=xsT_r)

    w_tiles = []
    for ki in range(KT):
        w_sb = wpool.tile(
            [128, OPC], mybir.dt.bfloat16, name=f"w_sb{it}_{ki}", tag="w_sb"
        )
        nc.sync.dma_start(out=w_sb[:], in_=wT_r[ki])
        w_tiles.append(w_sb)

    psums = [
        pspool.tile(
            [TOKENS, OBS], mybir.dt.float32, name=f"ps{it}_{ob}", tag=f"ps{ob}"
        )
        for ob in range(OB)
    ]

    for ki in range(KT):
        for ob in range(OB):
            nc.tensor.matmul(
                psums[ob][:, :],
                xs_sb[:, ki, :],
                w_tiles[ki][:, ob * OBS : (ob + 1) * OBS],
                start=(ki == 0),
                stop=(ki == KT - 1),
            )

    out_sb = opool.tile(
        [TOKENS, OPC], mybir.dt.float32, name=f"out_sb_{it}", tag="out_sb"
    )
    for ob in range(OB):
        nc.vector.tensor_copy(out_sb[:, ob * OBS : (ob + 1) * OBS], psums[ob][:, :])
    nc.sync.dma_start(out=out.ap(), in_=out_sb[:])


def _build(loops=1, hw_loop=False):
    """Build the per-core program. loops>1 repeats the whole body (hw_loop
    uses a Tile For_i so device time dominates the axon dispatch floor)."""
    key = (loops, hw_loop)
    if key in _cached_nc:
        return _cached_nc[key]

    nc = bacc.Bacc(
        "TRN2", target_bir_lowering=False, debug=False, num_devices=NCORES
    )
    xsT = nc.dram_tensor(
        "xsT", [IN_F, TOKENS], mybir.dt.bfloat16, kind="ExternalInput"
    )
    wT = nc.dram_tensor("wT", [IN_F, OPC], mybir.dt.bfloat16, kind="ExternalInput")
    out = nc.dram_tensor(
        "out", [TOKENS, OPC], mybir.dt.float32, kind="ExternalOutput"
    )

    xsT_r = xsT.ap().rearrange("(nk p) t -> p nk t", p=128)  # [128, 32, 32]
    wT_r = wT.ap().rearrange("(nk p) n -> nk p n", p=128)  # [32, 128, 1792]
    aps = (xsT_r, wT_r, out)

    with tile.TileContext(nc) as tc:
        with (
            tc.tile_pool(name="xpool", bufs=2) as xpool,
            tc.tile_pool(name="wpool", bufs=KT) as wpool,
            tc.tile_pool(name="opool", bufs=2) as opool,
            tc.tile_pool(name="pspool", bufs=2, space=bass.MemorySpace.PSUM) as pspool,
        ):
            pools = (xpool, wpool, opool, pspool)
            if hw_loop and loops > 1:
                with tc.For_i(0, loops, 1):
                    _emit_body(nc, tc, pools, aps)
            else:
                for it in range(loops):
                    _emit_body(nc, tc, pools, aps, it)

    nc.compile()
    _cached_nc[key] = nc
    return nc


def make_in_maps(x, weight, scales):
    x = np.asarray(x, dtype=np.float32)
    weight = np.asarray(weight)
    scales = np.asarray(scales, dtype=np.float32)
    assert x.shape == (TOKENS, IN_F) and weight.shape == (OUT_F, IN_F)

    xs = x * scales[None, :]
    xsT = np.ascontiguousarray(xs.T).astype(ml_dtypes.bfloat16)
    # int8-valued weights are exactly representable in bf16
    wT = weight.astype(np.float32).T  # [IN_F, OUT_F] view
    in_maps = []
    for c in range(NCORES):
        wTc = np.ascontiguousarray(wT[:, c * OPC : (c + 1) * OPC]).astype(
            ml_dtypes.bfloat16
        )
        in_maps.append({"xsT": xsT, "wT": wTc})
    return in_maps


def run(x, weight, scales, trace=False, trace_cores=None):
    nc = _build()
    in_maps = make_in_maps(x, weight, scales)
    res = run_bass_kernel_spmd(
        nc,
        in_maps,
        core_ids=list(range(NCORES)),
        trace=trace,
        trace_cores=trace_cores,
    )
    out = np.concatenate(
        [res.results[c]["out"] for c in range(NCORES)], axis=1
    ).astype(np.float32, copy=False)
    return out, res


def kernel(x, weight, scales):
    out, _ = run(x, weight, scales)
    return out
